# revision 11
# baseline (speedup 1.0000x reference)
"""Trainium2 Bass kernel for AdaptiveLiquidNetwork.

Reference computation (per full batch B=16384):
    projected  = tanh(x @ w_in.T + b_in)                     [B, U]
    A          = sigmoid(projected @ sensory_w + sigma)      [B, U]
    decay      = exp(-0.1 / tau)                             [U]
    new_states = A + (states - A) * decay                    [B, U]
    output     = new_states @ readout_w.T + readout_b        [B, D]

Strategy: data-parallel over 8 NeuronCores (2048 batch rows each),
weights replicated. On-chip dataflow keeps activations feature-major
([feature_part, batch_free]) so the contraction dim always sits on the
SBUF partition axis:
  - x arrives batch-major (f32) and is transposed on the TensorEngine
    directly from a stride-2 bf16 "high half" bitcast view of the f32
    data (bf16 == truncated f32), so the transposes run at bf16 rate
    with no separate cast pass. Two k-chunks of transposes share one
    PSUM bank ([128, 1024] bf16), halving the PSUM->SBUF copy count
    (copies alternate ScalarE/VectorE). The mm1 loop is k-pair-outer
    with 4 live PSUM accumulation groups and is software-pipelined:
    each pair's transposes+copy are emitted before the previous pair's
    matmuls, so PSUM-slot stalls at chunk boundaries overlap transpose
    work in the in-order PE stream.
  - mm1: projT[u, b] = w_inT[k, u] slices (stationary) x xT[k, b];
    tanh+bias fused into the ScalarEngine PSUM->SBUF activation
    (bias is per-partition in this orientation).
  - mm2: A_T[u', b] = sensory_w[u, u'] (natural layout!) x projT[u, b],
    sigmoid+bias fused the same way.
  - new_states: A_T is transposed back on the TensorEngine (cheap bf16
    128x128 identity transposes); the PSUM->SBUF copy doubles as the
    "*(1-decay)" multiply (zero-states fast path) via a broadcast tile
    on the VectorEngine.
  - mm3 is flipped: lhsT = A_T 128-column slices (already feature
    major), rhs = readout_w.T (pre-scaled by (1-decay) on host for the
    zero-states path) -> output lands batch-major in PSUM directly; the
    readout bias is added by the VectorEngine during the PSUM->SBUF
    copy with a broadcast bias tile.

All matmuls run in bf16 (fp32 PSUM accumulation): on TRN2 bf16 streams
one column/cycle like fp32, but LDWEIGHTS gets FWL + background-buffer
pull-ahead, which f32/f32r cannot use (their ~150ns weight load
serializes with every matmul). Outputs are written bf16 (halving output
DMA traffic) and upcast to f32 on the host; total error stays ~3e-3,
well inside the 2e-2 gate.

Startup choreography (the HAM clock gate keeps the PE at 1.2 GHz until
it has seen ~3.4us of sustained matmul activity, and re-throttles after
any ~3.4us matmul-free window): DMAs are issued in consumption order
(x chunk 0 first, then w_in as 4 k-pair slices, sensory, readout, then
the remaining x chunks all prefetched up front), and the initial DMA
wait is filled with dummy identity matmuls plus the broadcast-constant
outer products so the PE is already warm when the first real matmul
issues. The ScalarE tanh/sigmoid LUT loads are also forced early.

The [128]-wide broadcast tiles for readout_b and (1-decay) are built
on-chip with K=1 outer-product matmuls against a ones row (saves
~380KB of DMA), from a single [1, 1152] "smalls" input row.

Host-side prep is limited to weight re-layout/casting (transposes of
the small replicated [512,*] matrices, per-partition vector packing)
and the exp(-t/tau) scalar math; all O(B) work happens on-device.
"""

import os
import sys

import numpy as np

for _p in (
    "/opt/trn_rl_repo",
    os.path.expanduser("~/.axon_site"),
    os.path.expanduser("~/.axon_site/_ro/trn_rl_repo"),
    os.path.expanduser("~/.axon_site/_ro/pypackages"),
):
    if os.path.isdir(_p) and _p not in sys.path:
        sys.path.append(_p)

import ml_dtypes  # noqa: E402

import concourse.bass as bass  # noqa: E402
import concourse.tile as tile  # noqa: E402
from concourse import bacc, mybir  # noqa: E402
from concourse.bass_utils import run_bass_kernel_spmd  # noqa: E402

F32 = mybir.dt.float32
BF16 = mybir.dt.bfloat16
AF = mybir.ActivationFunctionType
NP_BF16 = ml_dtypes.bfloat16
OUT_DT = BF16  # outputs stored bf16, upcast to f32 on host

N_CORES = 8
B = 16384
D_IN = 1024
U = 512
D_OUT = 512
T_END = 0.1

BS = B // N_CORES          # batch rows per core (2048)
BCHUNK = 512               # batch rows per processing chunk
N_BC = BS // BCHUNK        # chunks per core (4)
N_BSUB = BCHUNK // 128     # 128-row subtiles per chunk (4)
N_KC = D_IN // 128         # contraction tiles for mm1 (8)
N_UC = U // 128            # feature tiles (4)


def _build(with_states: bool):
    nc = bacc.Bacc("TRN2", target_bir_lowering=False, debug=False)

    x = nc.dram_tensor("x", [BS, D_IN], F32, kind="ExternalInput").ap()
    w_inT = nc.dram_tensor("w_inT", [D_IN, U], BF16, kind="ExternalInput").ap()
    sensory = nc.dram_tensor("sensory_w", [U, U], BF16, kind="ExternalInput").ap()
    readout_wT = nc.dram_tensor("readout_wT", [U, D_OUT], BF16, kind="ExternalInput").ap()
    # vecs columns: [0:4]=b_in, [4:8]=sigma, [8:12]=1-decay, [12:16]=decay,
    # each packed [128, 4] with element [p, c] = v[c*128 + p].
    vecs = nc.dram_tensor("vecs", [128, 16], F32, kind="ExternalInput").ap()
    # smalls row: [0:128]=ones, [128:640]=readout_b, [640:1152]=1-decay
    smalls = nc.dram_tensor("smalls", [1, 1152], F32, kind="ExternalInput").ap()
    eye = nc.dram_tensor("eye128", [128, 128], BF16, kind="ExternalInput").ap()
    states = None
    eye32 = None
    if with_states:
        eye32 = nc.dram_tensor("eye128f", [128, 128], F32, kind="ExternalInput").ap()
        states = nc.dram_tensor("states", [BS, U], F32, kind="ExternalInput").ap()

    out = nc.dram_tensor("out", [BS, D_OUT], OUT_DT, kind="ExternalOutput").ap()
    new_states = nc.dram_tensor("new_states", [BS, U], OUT_DT, kind="ExternalOutput").ap()

    with tile.TileContext(nc) as tc:
        with (
            tc.tile_pool(name="const", bufs=1) as cpool,
            tc.tile_pool(name="xin0", bufs=2) as xpool0,
            tc.tile_pool(name="xin4", bufs=3) as xpool,
            tc.tile_pool(name="xt", bufs=4) as xtpool,
            tc.tile_pool(name="act", bufs=6) as apool,
            tc.tile_pool(name="onat", bufs=4) as opool,
            tc.tile_pool(name="pst", bufs=2, space="PSUM") as trppool,
            tc.tile_pool(name="psmm", bufs=4, space="PSUM") as mmppool,
            tc.tile_pool(name="psmm3", bufs=2, space="PSUM") as mm3ppool,
        ):
            # ---- DMA issue is expensive (~0.65us of engine-queue time per
            # DMA instruction) and both SP ("sync") and ACT ("scalar") have
            # hardware DGEs. The scalar queue wakes from the framework
            # preamble first, so it carries the critical path: the identity
            # (gates the PE warmup matmuls) and the big coalesced x-chunk
            # loads. The sync queue carries weights/constants in parallel. ----
            eye_sb = cpool.tile([128, 128], BF16, tag="eye")
            nc.scalar.dma_start(out=eye_sb[:], in_=eye[:])

            # x chunk 0 as two 2-subtile DMAs (finer arrival for the first
            # transposes), later chunks as one 4-subtile DMA each; returns
            # per-subtile AP views
            def load_x(bc):
                row0 = bc * BCHUNK
                if bc == 0:
                    views = []
                    for h in range(2):
                        t = xpool0.tile([128, 2 * D_IN], F32, tag="xa2")
                        nc.scalar.dma_start(
                            out=t[:].rearrange("p (i k) -> p i k", i=2),
                            in_=x[row0 + h * 256 : row0 + (h + 1) * 256, :].rearrange(
                                "(i p) k -> p i k", p=128
                            ),
                        )
                        views += [t[:, 0:D_IN], t[:, D_IN : 2 * D_IN]]
                    return views
                t = xpool.tile([128, N_BSUB * D_IN], F32, tag="xa4")
                nc.scalar.dma_start(
                    out=t[:].rearrange("p (i k) -> p i k", i=N_BSUB),
                    in_=x[row0 : row0 + BCHUNK, :].rearrange("(i p) k -> p i k", p=128),
                )
                return [t[:, i * D_IN : (i + 1) * D_IN] for i in range(N_BSUB)]

            # prefetch ALL x chunks up front (8MB total, fits SBUF)
            xall = [load_x(bc) for bc in range(N_BC)]

            vec_sb = cpool.tile([128, 16], F32, tag="vecs")
            nc.sync.dma_start(out=vec_sb[:], in_=vecs[:])
            sm_sb = cpool.tile([1, 1152], F32, tag="smalls")
            nc.sync.dma_start(out=sm_sb[:], in_=smalls[:])
            w_sb = cpool.tile([128, N_KC * U], BF16, tag="w_in")
            nc.sync.dma_start(
                out=w_sb[:].rearrange("p (kc u) -> p kc u", kc=N_KC),
                in_=w_inT.rearrange("(kc p) u -> p kc u", p=128),
            )
            ss_sb = cpool.tile([128, N_UC * U], BF16, tag="sensory")
            nc.sync.dma_start(
                out=ss_sb[:].rearrange("p (uc u) -> p uc u", uc=N_UC),
                in_=sensory.rearrange("(uc p) u -> p uc u", p=128),
            )
            rt_sb = cpool.tile([128, N_UC * D_OUT], BF16, tag="readout")
            nc.sync.dma_start(
                out=rt_sb[:].rearrange("p (uc d) -> p uc d", uc=N_UC),
                in_=readout_wT.rearrange("(uc p) d -> p uc d", p=128),
            )

            # dummy activations force the ScalarE LUT loads (~1.3us each)
            # to happen now, during the initial DMA wait, instead of right
            # before the first real tanh/sigmoid (after the scalar-queue
            # DMA issues: these block the scalar queue on the vecs DMA)
            warm = cpool.tile([1, 16], F32, tag="warm")
            nc.scalar.activation(warm[:1, :], vec_sb[:1, :], AF.Tanh)
            nc.scalar.activation(warm[:1, :], vec_sb[:1, :], AF.Sigmoid)

            # dummy matmuls on the identity tile: the PE HAM clock gate only
            # un-throttles after ~3.4us of CONTIGUOUS matmul activity (a full
            # 4096-cycle window with no holes -- transposes earn no credit),
            # so burn one dense burst during the startup DMA wait; after that
            # the gate stays warm as long as no ~3.4us matmul-free window
            # occurs, which the chunk pipeline guarantees
            def pe_warm(n):
                wp = mm3ppool.tile([128, 128], F32, tag="mm3", name=f"wps{nc.next_id()}")
                for _ in range(n):
                    nc.tensor.matmul(wp[:], lhsT=eye_sb[:], rhs=eye_sb[:])

            pe_warm(48)

            # broadcast constants built on-chip with K=1 outer products
            ones_r = sm_sb[:1, 0:128]
            ps_rb = mm3ppool.tile([128, D_OUT], F32, tag="mm3")
            nc.tensor.matmul(ps_rb[:], lhsT=ones_r, rhs=sm_sb[:1, 128:640])
            rb_sb = cpool.tile([128, D_OUT], F32, tag="rb")
            nc.vector.tensor_copy(rb_sb[:], ps_rb[:])
            ps_omd = mm3ppool.tile([128, U], F32, tag="mm3")
            nc.tensor.matmul(ps_omd[:], lhsT=ones_r, rhs=sm_sb[:1, 640:1152])
            omd_sb = cpool.tile([128, U], BF16, tag="omd")
            nc.vector.tensor_copy(omd_sb[:], ps_omd[:])

            if with_states:
                eye32_sb = cpool.tile([128, 128], F32, tag="eye32")
                nc.sync.dma_start(out=eye32_sb[:], in_=eye32[:])

            for bc in range(N_BC):
                row0 = bc * BCHUNK
                xa = xall[bc]

                # ---- mm1, kc-outer so the x transposes interleave with the
                # matmul stream (keeps the PE HAM clock gate warm): per
                # k-chunk, transpose 4 x subtiles into one PSUM bank, copy
                # them to SBUF (alternating ScalarE/VectorE, which also does
                # the f32->bf16 cast), then immediately accumulate that
                # k-chunk into all 4 uc PSUM groups. ----
                ps1 = [
                    mmppool.tile([128, BCHUNK], F32, tag="mm", name=f"ps1_{bc}_{uc}", bufs=4)
                    for uc in range(N_UC)
                ]
                # bf16 view of the high half of each f32 x element
                # (bf16 == truncated f32): the transposes then run at
                # 1 cycle/row with no separate cast pass
                xh = [
                    xa[i]
                    .bitcast(BF16)
                    .rearrange("p (k two) -> p k two", two=2)
                    for i in range(N_BSUB)
                ]
                def tr_group(kp):
                    # one PSUM bank holds the transposes of TWO k-chunks
                    # ([128, 1024] bf16 = exactly one bank), halving the
                    # PSUM->SBUF copy op count
                    pt = trppool.tile([128, 2 * BCHUNK], BF16, tag="tr")
                    for h in range(2):
                        kc = 2 * kp + h
                        for i in range(N_BSUB):
                            nc.tensor.transpose(
                                pt[:, h * BCHUNK + i * 128 : h * BCHUNK + (i + 1) * 128],
                                xh[i][:, kc * 128 : (kc + 1) * 128, 1],
                                eye_sb[:],
                            )
                    xt = xtpool.tile([128, 2 * BCHUNK], BF16, tag="xt")
                    if kp % 2 == 0:
                        nc.scalar.activation(xt[:], pt[:], AF.Copy)
                    else:
                        nc.vector.tensor_copy(xt[:], pt[:])
                    return xt

                def mm_group(kp, xt):
                    for h in range(2):
                        kc = 2 * kp + h
                        for uc in range(N_UC):
                            nc.tensor.matmul(
                                ps1[uc][:],
                                lhsT=w_sb[:, kc * U + uc * 128 : kc * U + (uc + 1) * 128],
                                rhs=xt[:, h * BCHUNK : (h + 1) * BCHUNK],
                                start=(kc == 0),
                                stop=(kc == N_KC - 1),
                            )

                # software-pipelined: each group's transposes+copy are
                # emitted BEFORE the previous group's matmuls, so a PSUM
                # slot stall at the chunk boundary overlaps transpose work
                # (the per-engine instruction stream is in-order)
                xt_prev = tr_group(0)
                for kp in range(1, N_KC // 2):
                    if bc == 0:
                        # plug HAM-credit holes while chunk 0 is DMA-paced
                        pe_warm(4)
                    xt_new = tr_group(kp)
                    mm_group(kp - 1, xt_prev)
                    xt_prev = xt_new
                mm_group(N_KC // 2 - 1, xt_prev)

                projT = []
                for uc in range(N_UC):
                    t = apool.tile([128, BCHUNK], BF16, tag="projT")
                    nc.scalar.activation(
                        t[:], ps1[uc][:], AF.Tanh, bias=vec_sb[:, uc : uc + 1]
                    )
                    projT.append(t)

                # ---- mm2 + sigmoid -> A_T[uc2] [128u', 512b] ----
                A_T = []
                for uc2 in range(N_UC):
                    ps = mmppool.tile([128, BCHUNK], F32, tag="mm")
                    for uc in range(N_UC):
                        nc.tensor.matmul(
                            ps[:],
                            lhsT=ss_sb[:, uc * U + uc2 * 128 : uc * U + (uc2 + 1) * 128],
                            rhs=projT[uc][:],
                            start=(uc == 0),
                            stop=(uc == N_UC - 1),
                        )
                    t = apool.tile([128, BCHUNK], BF16, tag="A_T")
                    nc.scalar.activation(
                        t[:], ps[:], AF.Sigmoid, bias=vec_sb[:, 4 + uc2 : 5 + uc2]
                    )
                    A_T.append(t)

                if not with_states:
                    # new_states = A * (1-decay); mm3 consumes A_T directly
                    # (readout_wT pre-scaled by (1-decay) on host).
                    nsT = A_T
                else:
                    # general path: new_states = A*(1-decay) + states*decay
                    st_nat = []
                    for i in range(N_BSUB):
                        t = xpool0.tile([128, U], F32, tag="st_nat", bufs=6)
                        nc.sync.dma_start(
                            out=t[:],
                            in_=states[row0 + i * 128 : row0 + (i + 1) * 128, :],
                        )
                        st_nat.append(t)
                    nsT = []
                    for uc2 in range(N_UC):
                        stT = xtpool.tile([128, BCHUNK], F32, tag="stT", bufs=2)
                        pt0 = trppool.tile([128, BCHUNK], F32, tag="tr")
                        for i in range(N_BSUB):
                            nc.tensor.transpose(
                                pt0[:, i * 128 : (i + 1) * 128],
                                st_nat[i][:, uc2 * 128 : (uc2 + 1) * 128],
                                eye32_sb[:],
                            )
                        nc.vector.tensor_copy(stT[:], pt0[:])
                        t1 = apool.tile([128, BCHUNK], F32, tag="ns_a", bufs=2)
                        nc.vector.tensor_scalar_mul(
                            t1[:], A_T[uc2][:], vec_sb[:, 8 + uc2 : 9 + uc2]
                        )
                        t2 = apool.tile([128, BCHUNK], F32, tag="ns_s", bufs=2)
                        nc.vector.tensor_scalar_mul(
                            t2[:], stT[:], vec_sb[:, 12 + uc2 : 13 + uc2]
                        )
                        t3 = apool.tile([128, BCHUNK], BF16, tag="nsT", bufs=6)
                        nc.vector.tensor_add(t3[:], t1[:], t2[:])
                        nsT.append(t3)

                last = bc == N_BC - 1

                # ---- epilogue: new_states back-transpose (+*(1-decay)) and
                # mm3 (batch-major) + bias. For the last chunk the two output
                # streams are interleaved per 128-row subtile with immediate
                # per-subtile stores, so the drain tail is as short as
                # possible; earlier chunks use two coalesced DMAs. ----
                nsn = opool.tile([128, N_BSUB * U], OUT_DT, tag="ns_nat", bufs=2)
                ob = opool.tile([128, N_BSUB * D_OUT], OUT_DT, tag="ob", bufs=2)

                def ns_subtile(i):
                    pt = trppool.tile([128, U], BF16, tag="tr")
                    for uc2 in range(N_UC):
                        nc.tensor.transpose(
                            pt[:, uc2 * 128 : (uc2 + 1) * 128],
                            nsT[uc2][:, i * 128 : (i + 1) * 128],
                            eye_sb[:],
                        )
                    if with_states:
                        nc.vector.tensor_copy(nsn[:, i * U : (i + 1) * U], pt[:])
                    else:
                        # fuse the *(1-decay) into the PSUM->SBUF copy
                        nc.vector.tensor_mul(
                            nsn[:, i * U : (i + 1) * U], pt[:], omd_sb[:]
                        )
                    if last:
                        nc.sync.dma_start(
                            out=new_states[row0 + i * 128 : row0 + (i + 1) * 128, :],
                            in_=nsn[:, i * U : (i + 1) * U],
                        )

                def mm3_subtile(i):
                    ps = mm3ppool.tile([128, D_OUT], F32, tag="mm3")
                    for uc2 in range(N_UC):
                        nc.tensor.matmul(
                            ps[:],
                            lhsT=nsT[uc2][:, i * 128 : (i + 1) * 128],
                            rhs=rt_sb[:, uc2 * D_OUT : (uc2 + 1) * D_OUT],
                            start=(uc2 == 0),
                            stop=(uc2 == N_UC - 1),
                        )
                    nc.vector.tensor_add(
                        ob[:, i * D_OUT : (i + 1) * D_OUT], ps[:], rb_sb[:]
                    )
                    if last:
                        nc.sync.dma_start(
                            out=out[row0 + i * 128 : row0 + (i + 1) * 128, :],
                            in_=ob[:, i * D_OUT : (i + 1) * D_OUT],
                        )

                if last:
                    for i in range(N_BSUB):
                        mm3_subtile(i)
                        ns_subtile(i)
                else:
                    for i in range(N_BSUB):
                        ns_subtile(i)
                    nc.sync.dma_start(
                        out=new_states[row0 : row0 + BCHUNK, :].rearrange(
                            "(i p) u -> p i u", p=128
                        ),
                        in_=nsn[:].rearrange("p (i u) -> p i u", i=N_BSUB),
                    )
                    for i in range(N_BSUB):
                        mm3_subtile(i)
                    nc.sync.dma_start(
                        out=out[row0 : row0 + BCHUNK, :].rearrange(
                            "(i p) d -> p i d", p=128
                        ),
                        in_=ob[:].rearrange("p (i d) -> p i d", i=N_BSUB),
                    )


    nc.compile()
    return nc


_GRAPHS: dict[bool, object] = {}


def _get_graph(with_states: bool):
    if with_states not in _GRAPHS:
        _GRAPHS[with_states] = _build(with_states)
    return _GRAPHS[with_states]


def _pack_cols(v):
    """[512] -> [128, 4] with [p, c] = v[c*128 + p]."""
    return np.ascontiguousarray(np.asarray(v, np.float32).reshape(4, 128).T)


def kernel(
    x,
    w_in,
    b_in,
    sensory_w,
    sensory_sigma,
    tau,
    readout_w,
    readout_b,
    states,
    _profile=False,
):
    x = np.ascontiguousarray(np.asarray(x, np.float32))
    w_in = np.asarray(w_in, np.float32)
    b_in = np.asarray(b_in, np.float32)
    sensory_w = np.asarray(sensory_w, np.float32)
    sensory_sigma = np.asarray(sensory_sigma, np.float32)
    tau = np.asarray(tau, np.float32)
    readout_w = np.asarray(readout_w, np.float32)
    readout_b = np.asarray(readout_b, np.float32)
    states = np.ascontiguousarray(np.asarray(states, np.float32))

    decay = np.exp(-T_END / tau).astype(np.float32)
    omd = (1.0 - decay).astype(np.float32)
    with_states = bool(states.any())

    w_inT = np.ascontiguousarray(w_in.T.astype(NP_BF16))
    rwT = readout_w.T.astype(np.float32)
    if not with_states:
        rwT = rwT * omd[:, None]
    readout_wT = np.ascontiguousarray(rwT.astype(NP_BF16))

    vecs = np.concatenate(
        [_pack_cols(b_in), _pack_cols(sensory_sigma), _pack_cols(omd), _pack_cols(decay)],
        axis=1,
    ).astype(np.float32)
    smalls = np.concatenate(
        [np.ones(128, np.float32), readout_b.astype(np.float32), omd]
    ).reshape(1, 1152)
    smalls = np.ascontiguousarray(smalls)
    eye = np.eye(128, dtype=NP_BF16)

    nc = _get_graph(with_states)

    in_maps = []
    for c in range(N_CORES):
        m = {
            "x": x[c * BS : (c + 1) * BS],
            "w_inT": w_inT,
            "sensory_w": np.ascontiguousarray(sensory_w.astype(NP_BF16)),
            "readout_wT": readout_wT,
            "vecs": vecs,
            "smalls": smalls,
            "eye128": eye,
        }
        if with_states:
            m["states"] = states[c * BS : (c + 1) * BS]
            m["eye128f"] = np.eye(128, dtype=np.float32)
        in_maps.append(m)

    res = run_bass_kernel_spmd(
        nc, in_maps, core_ids=list(range(N_CORES)), trace=_profile
    )

    out = np.concatenate(
        [res.results[c]["out"].astype(np.float32) for c in range(N_CORES)], axis=0
    )
    new_states = np.concatenate(
        [res.results[c]["new_states"].astype(np.float32) for c in range(N_CORES)],
        axis=0,
    )
    if _profile:
        return (out, new_states), res
    return (out, new_states)


# revision 12
# speedup vs baseline: 1.0045x; 1.0045x over previous
"""Trainium2 Bass kernel for AdaptiveLiquidNetwork.

Reference computation (per full batch B=16384):
    projected  = tanh(x @ w_in.T + b_in)                     [B, U]
    A          = sigmoid(projected @ sensory_w + sigma)      [B, U]
    decay      = exp(-0.1 / tau)                             [U]
    new_states = A + (states - A) * decay                    [B, U]
    output     = new_states @ readout_w.T + readout_b        [B, D]

Strategy: data-parallel over 8 NeuronCores (2048 batch rows each),
weights replicated. On-chip dataflow keeps activations feature-major
([feature_part, batch_free]) so the contraction dim always sits on the
SBUF partition axis:
  - x arrives batch-major (f32) and is transposed on the TensorEngine
    directly from a stride-2 bf16 "high half" bitcast view of the f32
    data (bf16 == truncated f32), so the transposes run at bf16 rate
    with no separate cast pass. Two k-chunks of transposes share one
    PSUM bank ([128, 1024] bf16), halving the PSUM->SBUF copy count
    (copies alternate ScalarE/VectorE). The mm1 loop is k-pair-outer
    with 4 live PSUM accumulation groups and is software-pipelined:
    each pair's transposes+copy are emitted before the previous pair's
    matmuls, so PSUM-slot stalls at chunk boundaries overlap transpose
    work in the in-order PE stream.
  - mm1: projT[u, b] = w_inT[k, u] slices (stationary) x xT[k, b];
    tanh+bias fused into the ScalarEngine PSUM->SBUF activation
    (bias is per-partition in this orientation).
  - mm2: A_T[u', b] = sensory_w[u, u'] (natural layout!) x projT[u, b],
    sigmoid+bias fused the same way.
  - new_states: A_T is transposed back on the TensorEngine (cheap bf16
    128x128 identity transposes); the PSUM->SBUF copy doubles as the
    "*(1-decay)" multiply (zero-states fast path) via a broadcast tile
    on the VectorEngine.
  - mm3 is flipped: lhsT = A_T 128-column slices (already feature
    major), rhs = readout_w.T (pre-scaled by (1-decay) on host for the
    zero-states path) -> output lands batch-major in PSUM directly; the
    readout bias is added by the VectorEngine during the PSUM->SBUF
    copy with a broadcast bias tile.

All matmuls run in bf16 (fp32 PSUM accumulation): on TRN2 bf16 streams
one column/cycle like fp32, but LDWEIGHTS gets FWL + background-buffer
pull-ahead, which f32/f32r cannot use (their ~150ns weight load
serializes with every matmul). Outputs are written bf16 (halving output
DMA traffic) and upcast to f32 on the host; total error stays ~3e-3,
well inside the 2e-2 gate.

Startup choreography (the HAM clock gate keeps the PE at 1.2 GHz until
it has seen ~3.4us of sustained matmul activity, and re-throttles after
any ~3.4us matmul-free window): DMAs are issued in consumption order
(x chunk 0 first, then w_in as 4 k-pair slices, sensory, readout, then
the remaining x chunks all prefetched up front), and the initial DMA
wait is filled with dummy identity matmuls plus the broadcast-constant
outer products so the PE is already warm when the first real matmul
issues. The ScalarE tanh/sigmoid LUT loads are also forced early.

The [128]-wide broadcast tiles for readout_b and (1-decay) are built
on-chip with K=1 outer-product matmuls against a ones row (saves
~380KB of DMA), from a single [1, 1152] "smalls" input row.

Host-side prep is limited to weight re-layout/casting (transposes of
the small replicated [512,*] matrices, per-partition vector packing)
and the exp(-t/tau) scalar math; all O(B) work happens on-device.
"""

import os
import sys

import numpy as np

for _p in (
    "/opt/trn_rl_repo",
    os.path.expanduser("~/.axon_site"),
    os.path.expanduser("~/.axon_site/_ro/trn_rl_repo"),
    os.path.expanduser("~/.axon_site/_ro/pypackages"),
):
    if os.path.isdir(_p) and _p not in sys.path:
        sys.path.append(_p)

import ml_dtypes  # noqa: E402

import concourse.bass as bass  # noqa: E402
import concourse.tile as tile  # noqa: E402
from concourse import bacc, mybir  # noqa: E402
from concourse.bass_utils import run_bass_kernel_spmd  # noqa: E402

F32 = mybir.dt.float32
BF16 = mybir.dt.bfloat16
AF = mybir.ActivationFunctionType
NP_BF16 = ml_dtypes.bfloat16
OUT_DT = BF16  # outputs stored bf16, upcast to f32 on host

N_CORES = 8
B = 16384
D_IN = 1024
U = 512
D_OUT = 512
T_END = 0.1

BS = B // N_CORES          # batch rows per core (2048)
BCHUNK = 512               # batch rows per processing chunk
N_BC = BS // BCHUNK        # chunks per core (4)
N_BSUB = BCHUNK // 128     # 128-row subtiles per chunk (4)
N_KC = D_IN // 128         # contraction tiles for mm1 (8)
N_UC = U // 128            # feature tiles (4)


def _build(with_states: bool):
    nc = bacc.Bacc("TRN2", target_bir_lowering=False, debug=False)

    x = nc.dram_tensor("x", [BS, D_IN], F32, kind="ExternalInput").ap()
    w_inT = nc.dram_tensor("w_inT", [D_IN, U], BF16, kind="ExternalInput").ap()
    sensory = nc.dram_tensor("sensory_w", [U, U], BF16, kind="ExternalInput").ap()
    readout_wT = nc.dram_tensor("readout_wT", [U, D_OUT], BF16, kind="ExternalInput").ap()
    # vecs columns: [0:4]=b_in, [4:8]=sigma, [8:12]=1-decay, [12:16]=decay,
    # each packed [128, 4] with element [p, c] = v[c*128 + p].
    vecs = nc.dram_tensor("vecs", [128, 16], F32, kind="ExternalInput").ap()
    # smalls row: [0:128]=ones, [128:640]=readout_b, [640:1152]=1-decay
    smalls = nc.dram_tensor("smalls", [1, 1152], F32, kind="ExternalInput").ap()
    eye = nc.dram_tensor("eye128", [128, 128], BF16, kind="ExternalInput").ap()
    states = None
    eye32 = None
    if with_states:
        eye32 = nc.dram_tensor("eye128f", [128, 128], F32, kind="ExternalInput").ap()
        states = nc.dram_tensor("states", [BS, U], F32, kind="ExternalInput").ap()

    out = nc.dram_tensor("out", [BS, D_OUT], OUT_DT, kind="ExternalOutput").ap()
    new_states = nc.dram_tensor("new_states", [BS, U], OUT_DT, kind="ExternalOutput").ap()

    with tile.TileContext(nc) as tc:
        with (
            tc.tile_pool(name="const", bufs=1) as cpool,
            tc.tile_pool(name="xin0", bufs=2) as xpool0,
            tc.tile_pool(name="xin4", bufs=3) as xpool,
            tc.tile_pool(name="xt", bufs=4) as xtpool,
            tc.tile_pool(name="act", bufs=6) as apool,
            tc.tile_pool(name="onat", bufs=4) as opool,
            tc.tile_pool(name="pst", bufs=2, space="PSUM") as trppool,
            tc.tile_pool(name="psmm", bufs=4, space="PSUM") as mmppool,
            tc.tile_pool(name="psmm3", bufs=2, space="PSUM") as mm3ppool,
        ):
            # ---- PE warmup with ZERO DMA dependencies: a gpsimd-memset
            # tile feeds dummy matmuls that start the moment the framework
            # preamble ends (~6us), so the HAM clock gate latches 2.4GHz
            # right as the first real data lands (the gate needs ~3.4us of
            # contiguous matmul credit; transposes earn none) ----
            wt_sb = cpool.tile([128, 128], BF16, tag="warmmm")
            nc.gpsimd.memset(wt_sb[:], 0.0)

            def pe_warm(n):
                wp = mm3ppool.tile([128, 128], F32, tag="mm3", name=f"wps{nc.next_id()}")
                for _ in range(n):
                    nc.tensor.matmul(wp[:], lhsT=wt_sb[:], rhs=wt_sb[:])

            pe_warm(34)

            # ---- DMA issue costs ~0.7us of queue time per instruction, so
            # both hardware DGE queues are used: the scalar (ACT) queue
            # carries the x path (it wakes from the preamble first), the
            # sync (SP) queue carries weights/constants. ----
            eye_sb = cpool.tile([128, 128], BF16, tag="eye")
            nc.scalar.dma_start(out=eye_sb[:], in_=eye[:])

            # x chunk 0 split per-subtile (finest possible start), chunks
            # 1-3 coalesced into one DMA each; returns per-subtile views
            def load_x(bc):
                row0 = bc * BCHUNK
                if bc == 0:
                    views = []
                    for i in range(2):
                        t = xpool0.tile([128, D_IN], F32, tag="xa1", bufs=2)
                        nc.scalar.dma_start(
                            out=t[:], in_=x[row0 + i * 128 : row0 + (i + 1) * 128, :]
                        )
                        views.append(t[:, :])
                    t = xpool0.tile([128, 2 * D_IN], F32, tag="xa2", bufs=1)
                    nc.scalar.dma_start(
                        out=t[:].rearrange("p (i k) -> p i k", i=2),
                        in_=x[row0 + 256 : row0 + 512, :].rearrange(
                            "(i p) k -> p i k", p=128
                        ),
                    )
                    views += [t[:, 0:D_IN], t[:, D_IN : 2 * D_IN]]
                    return views
                t = xpool.tile([128, N_BSUB * D_IN], F32, tag="xa4")
                nc.scalar.dma_start(
                    out=t[:].rearrange("p (i k) -> p i k", i=N_BSUB),
                    in_=x[row0 : row0 + BCHUNK, :].rearrange("(i p) k -> p i k", p=128),
                )
                return [t[:, i * D_IN : (i + 1) * D_IN] for i in range(N_BSUB)]

            xall = [load_x(bc) for bc in range(N_BC)]

            # sync queue: w_in first (its kc slices gate the first matmuls),
            # split in two halves so half 0 arrives before subtile 0's mm1
            w_half = []
            for h in range(2):
                t = cpool.tile([128, 4 * U], BF16, tag=f"w_in{h}")
                nc.sync.dma_start(
                    out=t[:].rearrange("p (kc u) -> p kc u", kc=4),
                    in_=w_inT.rearrange("(kc p) u -> p kc u", p=128)[
                        :, 4 * h : 4 * h + 4, :
                    ],
                )
                w_half.append(t)

            def w_slice(kc, uc):
                t = w_half[kc // 4]
                k = kc % 4
                return t[:, k * U + uc * 128 : k * U + (uc + 1) * 128]

            vec_sb = cpool.tile([128, 16], F32, tag="vecs")
            nc.sync.dma_start(out=vec_sb[:], in_=vecs[:])
            sm_sb = cpool.tile([1, 1152], F32, tag="smalls")
            nc.sync.dma_start(out=sm_sb[:], in_=smalls[:])
            ss_sb = cpool.tile([128, N_UC * U], BF16, tag="sensory")
            nc.sync.dma_start(
                out=ss_sb[:].rearrange("p (uc u) -> p uc u", uc=N_UC),
                in_=sensory.rearrange("(uc p) u -> p uc u", p=128),
            )
            rt_sb = cpool.tile([128, N_UC * D_OUT], BF16, tag="readout")
            nc.sync.dma_start(
                out=rt_sb[:].rearrange("p (uc d) -> p uc d", uc=N_UC),
                in_=readout_wT.rearrange("(uc p) d -> p uc d", p=128),
            )

            # dummy activations force the ScalarE LUT loads (~1.3us each)
            # to happen during the initial DMA wait (issued after the
            # scalar-queue DMAs so they don't delay them)
            warm = cpool.tile([1, 16], F32, tag="warm")
            nc.scalar.activation(warm[:1, :], vec_sb[:1, :], AF.Tanh)
            nc.scalar.activation(warm[:1, :], vec_sb[:1, :], AF.Sigmoid)

            # broadcast constants built on-chip with K=1 outer products
            # (more matmul warmup credit, saves ~380KB of DMA)
            ones_r = sm_sb[:1, 0:128]
            ps_rb = mm3ppool.tile([128, D_OUT], F32, tag="mm3")
            nc.tensor.matmul(ps_rb[:], lhsT=ones_r, rhs=sm_sb[:1, 128:640])
            rb_sb = cpool.tile([128, D_OUT], F32, tag="rb")
            nc.vector.tensor_copy(rb_sb[:], ps_rb[:])
            ps_omd = mm3ppool.tile([128, U], F32, tag="mm3")
            nc.tensor.matmul(ps_omd[:], lhsT=ones_r, rhs=sm_sb[:1, 640:1152])
            omd_sb = cpool.tile([128, U], BF16, tag="omd")
            nc.vector.tensor_copy(omd_sb[:], ps_omd[:])

            if with_states:
                eye32_sb = cpool.tile([128, 128], F32, tag="eye32")
                nc.sync.dma_start(out=eye32_sb[:], in_=eye32[:])

            for bc in range(N_BC):
                row0 = bc * BCHUNK
                xa = xall[bc]

                ps1 = [
                    mmppool.tile([128, BCHUNK], F32, tag="mm", name=f"ps1_{bc}_{uc}", bufs=4)
                    for uc in range(N_UC)
                ]
                # bf16 view of the high half of each f32 x element
                # (bf16 == truncated f32): the transposes then run at
                # 1 cycle/row with no separate cast pass
                xh = [
                    xa[i]
                    .bitcast(BF16)
                    .rearrange("p (k two) -> p k two", two=2)
                    for i in range(N_BSUB)
                ]

                if bc == 0:
                    # chunk 0 is paced by x arrival: process per 128-row
                    # subtile (N=128 matmuls) so the PE runs densely from
                    # the moment the first 512KB subtile lands, instead of
                    # idling for the whole 2MB chunk (idle > ~3.4us would
                    # re-throttle the clock gate mid-kernel)
                    for i in range(N_BSUB):
                        pt = trppool.tile([128, N_KC * 128], BF16, tag="tr")
                        for kc in range(N_KC):
                            nc.tensor.transpose(
                                pt[:, kc * 128 : (kc + 1) * 128],
                                xh[i][:, kc * 128 : (kc + 1) * 128, 1],
                                eye_sb[:],
                            )
                        xt = xtpool.tile([128, N_KC * 128], BF16, tag="xt")
                        if i % 2 == 0:
                            nc.scalar.activation(xt[:], pt[:], AF.Copy)
                        else:
                            nc.vector.tensor_copy(xt[:], pt[:])
                        for kc in range(N_KC):
                            for uc in range(N_UC):
                                nc.tensor.matmul(
                                    ps1[uc][:, i * 128 : (i + 1) * 128],
                                    lhsT=w_slice(kc, uc),
                                    rhs=xt[:, kc * 128 : (kc + 1) * 128],
                                    start=(kc == 0),
                                    stop=(kc == N_KC - 1),
                                )
                else:
                    # ---- mm1, k-pair-outer, software-pipelined with the
                    # transposes (PSUM-slot stalls overlap transpose work) ----
                    def tr_group(kp):
                        # one PSUM bank holds the transposes of TWO k-chunks
                        pt = trppool.tile([128, 2 * BCHUNK], BF16, tag="tr")
                        for h in range(2):
                            kc = 2 * kp + h
                            for i in range(N_BSUB):
                                nc.tensor.transpose(
                                    pt[:, h * BCHUNK + i * 128 : h * BCHUNK + (i + 1) * 128],
                                    xh[i][:, kc * 128 : (kc + 1) * 128, 1],
                                    eye_sb[:],
                                )
                        xt = xtpool.tile([128, 2 * BCHUNK], BF16, tag="xt")
                        if kp % 2 == 0:
                            nc.scalar.activation(xt[:], pt[:], AF.Copy)
                        else:
                            nc.vector.tensor_copy(xt[:], pt[:])
                        return xt

                    def mm_group(kp, xt):
                        for h in range(2):
                            kc = 2 * kp + h
                            for uc in range(N_UC):
                                nc.tensor.matmul(
                                    ps1[uc][:],
                                    lhsT=w_slice(kc, uc),
                                    rhs=xt[:, h * BCHUNK : (h + 1) * BCHUNK],
                                    start=(kc == 0),
                                    stop=(kc == N_KC - 1),
                                )

                    xt_prev = tr_group(0)
                    for kp in range(1, N_KC // 2):
                        xt_new = tr_group(kp)
                        mm_group(kp - 1, xt_prev)
                        xt_prev = xt_new
                    mm_group(N_KC // 2 - 1, xt_prev)

                projT = []
                for uc in range(N_UC):
                    t = apool.tile([128, BCHUNK], BF16, tag="projT")
                    nc.scalar.activation(
                        t[:], ps1[uc][:], AF.Tanh, bias=vec_sb[:, uc : uc + 1]
                    )
                    projT.append(t)

                # ---- mm2 + sigmoid -> A_T[uc2] [128u', 512b] ----
                A_T = []
                for uc2 in range(N_UC):
                    ps = mmppool.tile([128, BCHUNK], F32, tag="mm")
                    for uc in range(N_UC):
                        nc.tensor.matmul(
                            ps[:],
                            lhsT=ss_sb[:, uc * U + uc2 * 128 : uc * U + (uc2 + 1) * 128],
                            rhs=projT[uc][:],
                            start=(uc == 0),
                            stop=(uc == N_UC - 1),
                        )
                    t = apool.tile([128, BCHUNK], BF16, tag="A_T")
                    nc.scalar.activation(
                        t[:], ps[:], AF.Sigmoid, bias=vec_sb[:, 4 + uc2 : 5 + uc2]
                    )
                    A_T.append(t)

                if not with_states:
                    # new_states = A * (1-decay); mm3 consumes A_T directly
                    # (readout_wT pre-scaled by (1-decay) on host).
                    nsT = A_T
                else:
                    # general path: new_states = A*(1-decay) + states*decay
                    st_nat = []
                    for i in range(N_BSUB):
                        t = xpool0.tile([128, U], F32, tag="st_nat", bufs=6)
                        nc.sync.dma_start(
                            out=t[:],
                            in_=states[row0 + i * 128 : row0 + (i + 1) * 128, :],
                        )
                        st_nat.append(t)
                    nsT = []
                    for uc2 in range(N_UC):
                        stT = xtpool.tile([128, BCHUNK], F32, tag="stT", bufs=2)
                        pt0 = trppool.tile([128, BCHUNK], F32, tag="tr")
                        for i in range(N_BSUB):
                            nc.tensor.transpose(
                                pt0[:, i * 128 : (i + 1) * 128],
                                st_nat[i][:, uc2 * 128 : (uc2 + 1) * 128],
                                eye32_sb[:],
                            )
                        nc.vector.tensor_copy(stT[:], pt0[:])
                        t1 = apool.tile([128, BCHUNK], F32, tag="ns_a", bufs=2)
                        nc.vector.tensor_scalar_mul(
                            t1[:], A_T[uc2][:], vec_sb[:, 8 + uc2 : 9 + uc2]
                        )
                        t2 = apool.tile([128, BCHUNK], F32, tag="ns_s", bufs=2)
                        nc.vector.tensor_scalar_mul(
                            t2[:], stT[:], vec_sb[:, 12 + uc2 : 13 + uc2]
                        )
                        t3 = apool.tile([128, BCHUNK], BF16, tag="nsT", bufs=6)
                        nc.vector.tensor_add(t3[:], t1[:], t2[:])
                        nsT.append(t3)

                last = bc == N_BC - 1

                # ---- epilogue: new_states back-transpose (+*(1-decay)) and
                # mm3 (batch-major) + bias. Output stores are split across
                # both DGE queues (ns on scalar, out on sync) because DMA
                # issue serializes at ~0.7us per instruction per queue; the
                # last chunk stores in row halves so the drain starts while
                # the second half still computes. ----
                nsn = opool.tile([128, N_BSUB * U], OUT_DT, tag="ns_nat", bufs=2)
                ob = opool.tile([128, N_BSUB * D_OUT], OUT_DT, tag="ob", bufs=2)

                def ns_subtile(i):
                    pt = trppool.tile([128, U], BF16, tag="tr")
                    for uc2 in range(N_UC):
                        nc.tensor.transpose(
                            pt[:, uc2 * 128 : (uc2 + 1) * 128],
                            nsT[uc2][:, i * 128 : (i + 1) * 128],
                            eye_sb[:],
                        )
                    if with_states:
                        nc.vector.tensor_copy(nsn[:, i * U : (i + 1) * U], pt[:])
                    else:
                        # fuse the *(1-decay) into the PSUM->SBUF copy
                        nc.vector.tensor_mul(
                            nsn[:, i * U : (i + 1) * U], pt[:], omd_sb[:]
                        )

                def mm3_subtile(i):
                    ps = mm3ppool.tile([128, D_OUT], F32, tag="mm3")
                    for uc2 in range(N_UC):
                        nc.tensor.matmul(
                            ps[:],
                            lhsT=nsT[uc2][:, i * 128 : (i + 1) * 128],
                            rhs=rt_sb[:, uc2 * D_OUT : (uc2 + 1) * D_OUT],
                            start=(uc2 == 0),
                            stop=(uc2 == N_UC - 1),
                        )
                    nc.vector.tensor_add(
                        ob[:, i * D_OUT : (i + 1) * D_OUT], ps[:], rb_sb[:]
                    )

                def store_half(h):
                    r0 = row0 + h * 256
                    nc.scalar.dma_start(
                        out=new_states[r0 : r0 + 256, :].rearrange(
                            "(i p) u -> p i u", p=128
                        ),
                        in_=nsn[:, 2 * h * U : (2 * h + 2) * U].rearrange(
                            "p (i u) -> p i u", i=2
                        ),
                    )
                    nc.sync.dma_start(
                        out=out[r0 : r0 + 256, :].rearrange("(i p) d -> p i d", p=128),
                        in_=ob[:, 2 * h * D_OUT : (2 * h + 2) * D_OUT].rearrange(
                            "p (i d) -> p i d", i=2
                        ),
                    )

                if last:
                    for h in range(2):
                        for i in (2 * h, 2 * h + 1):
                            mm3_subtile(i)
                            ns_subtile(i)
                        store_half(h)
                else:
                    for i in range(N_BSUB):
                        ns_subtile(i)
                    nc.scalar.dma_start(
                        out=new_states[row0 : row0 + BCHUNK, :].rearrange(
                            "(i p) u -> p i u", p=128
                        ),
                        in_=nsn[:].rearrange("p (i u) -> p i u", i=N_BSUB),
                    )
                    for i in range(N_BSUB):
                        mm3_subtile(i)
                    nc.sync.dma_start(
                        out=out[row0 : row0 + BCHUNK, :].rearrange(
                            "(i p) d -> p i d", p=128
                        ),
                        in_=ob[:].rearrange("p (i d) -> p i d", i=N_BSUB),
                    )


    nc.compile()
    return nc


_GRAPHS: dict[bool, object] = {}


def _get_graph(with_states: bool):
    if with_states not in _GRAPHS:
        _GRAPHS[with_states] = _build(with_states)
    return _GRAPHS[with_states]


def _pack_cols(v):
    """[512] -> [128, 4] with [p, c] = v[c*128 + p]."""
    return np.ascontiguousarray(np.asarray(v, np.float32).reshape(4, 128).T)


def kernel(
    x,
    w_in,
    b_in,
    sensory_w,
    sensory_sigma,
    tau,
    readout_w,
    readout_b,
    states,
    _profile=False,
):
    x = np.ascontiguousarray(np.asarray(x, np.float32))
    w_in = np.asarray(w_in, np.float32)
    b_in = np.asarray(b_in, np.float32)
    sensory_w = np.asarray(sensory_w, np.float32)
    sensory_sigma = np.asarray(sensory_sigma, np.float32)
    tau = np.asarray(tau, np.float32)
    readout_w = np.asarray(readout_w, np.float32)
    readout_b = np.asarray(readout_b, np.float32)
    states = np.ascontiguousarray(np.asarray(states, np.float32))

    decay = np.exp(-T_END / tau).astype(np.float32)
    omd = (1.0 - decay).astype(np.float32)
    with_states = bool(states.any())

    w_inT = np.ascontiguousarray(w_in.T.astype(NP_BF16))
    rwT = readout_w.T.astype(np.float32)
    if not with_states:
        rwT = rwT * omd[:, None]
    readout_wT = np.ascontiguousarray(rwT.astype(NP_BF16))

    vecs = np.concatenate(
        [_pack_cols(b_in), _pack_cols(sensory_sigma), _pack_cols(omd), _pack_cols(decay)],
        axis=1,
    ).astype(np.float32)
    smalls = np.concatenate(
        [np.ones(128, np.float32), readout_b.astype(np.float32), omd]
    ).reshape(1, 1152)
    smalls = np.ascontiguousarray(smalls)
    eye = np.eye(128, dtype=NP_BF16)

    nc = _get_graph(with_states)

    in_maps = []
    for c in range(N_CORES):
        m = {
            "x": x[c * BS : (c + 1) * BS],
            "w_inT": w_inT,
            "sensory_w": np.ascontiguousarray(sensory_w.astype(NP_BF16)),
            "readout_wT": readout_wT,
            "vecs": vecs,
            "smalls": smalls,
            "eye128": eye,
        }
        if with_states:
            m["states"] = states[c * BS : (c + 1) * BS]
            m["eye128f"] = np.eye(128, dtype=np.float32)
        in_maps.append(m)

    res = run_bass_kernel_spmd(
        nc, in_maps, core_ids=list(range(N_CORES)), trace=_profile
    )

    out = np.concatenate(
        [res.results[c]["out"].astype(np.float32) for c in range(N_CORES)], axis=0
    )
    new_states = np.concatenate(
        [res.results[c]["new_states"].astype(np.float32) for c in range(N_CORES)],
        axis=0,
    )
    if _profile:
        return (out, new_states), res
    return (out, new_states)
